# revision 72
# baseline (speedup 1.0000x reference)
"""Causal MHA on 8 trn2 NeuronCores.

Sharding: core c handles batch b = c // 4 and head group g = c % 4
(heads 4g..4g+3).  Megatron-style TP: W_kqv column-split per head
group, W_proj row-split; the row-parallel all-reduce (sum of the 4
head-group partials per batch) happens on the host at gather time.

Per-core program (bf16 matmul operands, fp32 PSUM accumulation):
  - all DRAM inputs are pre-tiled on the host to the exact SBUF layouts
    (one contiguous run per partition) so every load is a single cheap
    DMA trigger, split across the two HWDGE queues (SP + ACT engine).
  - qT,kT produced directly in [feat, T] layout (lhsT=W tiles, rhs=xT
    tiles), v in [T, feat] layout, so no on-device transposes.
  - scores computed transposed, sT[k,q]; two heads packed into the PE
    array rows (K=64 each) via base-partition 0/64 -> concurrent MMs
    into the two banks of one [128,2,512] f32 PSUM tile.
  - causal trim: diagonal k-tiles only compute the valid q-range
    (N = 512-128m), so scores/exp/PV skip ~15% of work; the causal
    boundary is a [128,2,128] triangular-mask multiply (c >= p) per
    diagonal step, and PV of the already-valid region runs straight
    off exp so the mask never gates the bulk of the work.
  - exp on the scalar engine straight out of PSUM, one instruction
    covering both heads (1/sqrt(hd) folded into Wq on the host).
  - PV uses ones-augmented V ([k,65] lhsT) so PSUM row 64 accumulates
    the softmax denominator Z alongside the 64 output dims.
  - normalization per head-pair (pair0's runs under pair1's attention):
    Z rows evacuate via ACT/DVE split copies, 1/Z via the fast DVE
    reciprocal approximation, broadcast across 64 partitions with a
    tiny sel-matmul, then one DVE multiply per half.
  - projection of pair0's features for the last q-tile runs during
    pair1's attention (per-ch split with a DVE add), shrinking the tail.
  - emission interleaves QKV(j+1) and proj(j-1) matmuls into
    attention(j)'s exp-gated stream so the PE stays dense and warm;
    activations are per-j tiles so interleaved phases share no tiles.
  - the v-bias matmul is skipped when b_kqv is all-zero (it is, for
    this model); a with_bias program variant handles the general case.
"""

import sys

sys.path.insert(0, "/opt/trn_rl_repo")

import ml_dtypes
import numpy as np

import concourse.bass as bass
import concourse.tile as tile
from concourse import bacc, mybir

F32 = mybir.dt.float32
F32R = mybir.dt.float32r
BF16 = mybir.dt.bfloat16

B, T, D = 2, 2048, 1024
H, HD = 16, 64
N_CORES = 8
HPG = H // (N_CORES // B)  # heads per group = 4
GF = HPG * HD  # per-group feature width = 256
DT = 512  # t/q tile width
KT = 128  # k tile width
NJ = T // DT  # 4
ND = D // 128  # 8 contraction chunks

Exp = mybir.ActivationFunctionType.Exp
Ln = mybir.ActivationFunctionType.Ln


def build_program(num_devices=N_CORES, with_bias=False):
    nc = bacc.Bacc(
        "TRN2", target_bir_lowering=False, debug=False, num_devices=num_devices
    )
    # inputs are pre-tiled on the host to the exact SBUF layouts so each
    # load is ONE dma trigger with one contiguous run per partition
    # (HWDGE descriptor generation costs ~5ns/descriptor; 128 descs/DMA)
    xT_d = nc.dram_tensor("xT", [128, NJ, ND, DT], BF16, kind="ExternalInput")
    wq_d = nc.dram_tensor("wq", [128, 2, ND, 128], BF16, kind="ExternalInput")
    wk_d = nc.dram_tensor("wk", [128, 2, ND, 128], BF16, kind="ExternalInput")
    wv_d = nc.dram_tensor("wv", [128, ND, GF], BF16, kind="ExternalInput")
    wp_d = nc.dram_tensor("wp", [128, 2, D], BF16, kind="ExternalInput")
    bq_d = nc.dram_tensor("bq", [128, 2], F32, kind="ExternalInput")
    bk_d = nc.dram_tensor("bk", [128, 2], F32, kind="ExternalInput")
    bv_d = nc.dram_tensor("bv", [1, GF], BF16, kind="ExternalInput")
    ones_d = nc.dram_tensor("ones", [128, 128], BF16, kind="ExternalInput")
    msk_d = nc.dram_tensor("msk", [128, 2, KT], BF16, kind="ExternalInput")
    sel_d = nc.dram_tensor("sel", [128, 256], BF16, kind="ExternalInput")
    y_d = nc.dram_tensor("y", [T, D], BF16, kind="ExternalOutput")

    with tile.TileContext(nc) as tc:
        with (
            tc.tile_pool(name="singles", bufs=1) as singles,
            tc.tile_pool(name="ea", bufs=6) as e_pool,
            tc.tile_pool(name="rz", bufs=2) as rz_pool,
            tc.tile_pool(name="ysb", bufs=8) as y_pool,
            tc.tile_pool(name="tr", bufs=2, space="PSUM") as tr_pool,
            tc.tile_pool(name="sc", bufs=1, space="PSUM") as sc_pool,
            tc.tile_pool(name="pv", bufs=2, space="PSUM") as pv_pool,
        ):
            # ---- weights / constants resident in SBUF ----
            wq_sb = singles.tile([128, 2, ND, 128], BF16)
            wk_sb = singles.tile([128, 2, ND, 128], BF16)
            wv_sb = singles.tile([128, ND, GF], BF16)
            wp_sb = singles.tile([128, 2, D], BF16)
            bq_sb = singles.tile([128, 2], F32)
            bk_sb = singles.tile([128, 2], F32)
            bv_sb = singles.tile([1, GF], BF16)
            ones_sb = singles.tile([128, 128], BF16)
            msk_sb = singles.tile([128, 2, KT], BF16)
            sel_sb = singles.tile([128, 256], BF16)

            # per-j activation tiles (distinct tiles -> no false deps
            # between interleaved phases)
            qT_t = [singles.tile([128, 2, DT], BF16, tag=f"qT{j}", name=f"qT{j}") for j in range(NJ)]
            kT_t = [singles.tile([128, 2, DT], BF16, tag=f"kT{j}", name=f"kT{j}") for j in range(NJ)]
            v_t = [
                singles.tile([128, DT // KT, HPG, HD + 1], BF16, tag=f"v{j}", name=f"v{j}")
                for j in range(NJ)
            ]
            o_t = [singles.tile([128, 2, DT], BF16, tag=f"oT{j}", name=f"oT{j}") for j in range(NJ)]
            xt_t = [
                singles.tile([128, ND, DT], BF16, tag=f"xt{j}", name=f"xt{j}")
                for j in range(NJ)
            ]

            # ---- startup loads split across the two HWDGE queues (SP +
            # Activation engine): x tiles on SP, weights on ACT; the first
            # QKV accumulation starts once xt0's first half + wq land ----
            nc.sync.dma_start(xt_t[0][:, 0:4], xT_d.ap()[:, 0, 0:4]).annotate("ld:xt0")
            nc.sync.dma_start(xt_t[0][:, 4:8], xT_d.ap()[:, 0, 4:8]).annotate("ld:xt0")
            nc.scalar.dma_start(wq_sb[:, 0], wq_d.ap()[:, 0]).annotate("ld:wq")
            nc.scalar.dma_start(wk_sb[:, 0], wk_d.ap()[:, 0]).annotate("ld:wk")
            nc.scalar.dma_start(wq_sb[:, 1], wq_d.ap()[:, 1]).annotate("ld:wq")
            nc.scalar.dma_start(wk_sb[:, 1], wk_d.ap()[:, 1]).annotate("ld:wk")
            nc.scalar.dma_start(wv_sb, wv_d.ap()).annotate("ld:wv")
            nc.sync.dma_start(bq_sb, bq_d.ap()).annotate("ld:b")
            nc.sync.dma_start(bk_sb, bk_d.ap()).annotate("ld:b")
            nc.sync.dma_start(bv_sb, bv_d.ap()).annotate("ld:b")
            nc.sync.dma_start(ones_sb, ones_d.ap()).annotate("ld:b")
            nc.sync.dma_start(msk_sb, msk_d.ap()).annotate("ld:b")
            nc.sync.dma_start(sel_sb, sel_d.ap()).annotate("ld:b")
            nc.scalar.dma_start(xt_t[1], xT_d.ap()[:, 1]).annotate("ld:xt1")
            for j in range(2, NJ):
                nc.sync.dma_start(xt_t[j], xT_d.ap()[:, j]).annotate(f"ld:xt{j}")
            nc.scalar.dma_start(wp_sb, wp_d.ap()).annotate("ld:wp")
            ones1 = ones_sb[0:1, :]
            for j in range(NJ):
                nc.vector.tensor_copy(
                    out=v_t[j][:, :, :, HD],
                    in_=ones_sb[:, 0 : DT // KT * HPG].rearrange(
                        "p (a b) -> p a b", a=DT // KT
                    ),
                ).annotate("v:ones")

            def qkv_closures(j):
                """QKV production for t-tile j as a list of closures (q/k
                split per ch so pair0's attention can start after 2)."""
                cls = []
                for ch in range(2):
                    for w_sb, b_sb, dst in (
                        (wq_sb, bq_sb, qT_t[j]),
                        (wk_sb, bk_sb, kT_t[j]),
                    ):

                        def qk(j=j, ch=ch, w_sb=w_sb, b_sb=b_sb, dst=dst):
                            ps = tr_pool.tile([128, DT], F32, tag="tr")
                            for d in range(ND):
                                nc.tensor.matmul(
                                    ps,
                                    w_sb[:, ch, d, :],
                                    xt_t[j][:, d, :],
                                    start=(d == 0),
                                    stop=(d == ND - 1),
                                ).annotate("mm:qk")
                            nc.vector.tensor_scalar_add(
                                out=dst[:, ch, :],
                                in0=ps,
                                scalar1=b_sb[:, ch : ch + 1],
                            ).annotate("cp:qk")

                        cls.append(qk)

                for t_ in range(DT // KT):

                    def vv(j=j, t_=t_):
                        ps = tr_pool.tile([128, DT], F32, tag="tr")
                        ssl = slice(t_ * 128, t_ * 128 + 128)
                        for d in range(ND):
                            nc.tensor.matmul(
                                ps[:, 0:GF],
                                xt_t[j][:, d, ssl],
                                wv_sb[:, d, :],
                                start=(d == 0),
                                stop=(not with_bias) and (d == ND - 1),
                            ).annotate("mm:v")
                        if with_bias:
                            nc.tensor.matmul(
                                ps[:, 0:GF], ones1, bv_sb, start=False, stop=True
                            ).annotate("mm:vb")
                        nc.scalar.copy(
                            out=v_t[j][:, t_, :, 0:HD],
                            in_=ps[:, 0:GF].rearrange("p (h c) -> p h c", c=HD),
                        ).annotate("cp:v")

                    cls.append(vv)
                return cls

            def attn_closures(j):
                """Attention for q-tile j: per-(pair,kt) closures plus a
                normalize closure per pair."""
                nk = 4 * (j + 1)
                cls = []
                zz = rz_pool.tile([128, DT], F32, tag="zz", name=f"zz{j}")
                nc.vector.memset(zz, 1.0)
                for pair in range(2):
                    pvA = pv_pool.tile([HD + 1, DT], F32, tag="pv")
                    pvB = pv_pool.tile([HD + 1, DT], F32, tag="pv")

                    for kt0 in range(0, nk, 2):

                        def step2(j=j, pair=pair, kt0=kt0, pvA=pvA, pvB=pvB,
                                  nk=nk):
                            # two consecutive k-steps share one 4-bank PSUM
                            # tile; off-diagonal pairs get ONE exp over all
                            # four banks (halving ACT instruction overhead)
                            psc = sc_pool.tile([128, 4, DT], F32, tag="sc")
                            e = e_pool.tile([128, 4, DT], BF16, tag="e")
                            pvs = (pvA, pvB)
                            for s_ in range(2):
                                kt = kt0 + s_
                                m = kt - 4 * j
                                qoff = 128 * m if m >= 0 else 0
                                jk, km = kt // (DT // KT), kt % (DT // KT)
                                ksl = slice(km * KT, km * KT + KT)
                                for h in range(2):
                                    nc.tensor.matmul(
                                        psc[:, 2 * s_ + h, qoff:],
                                        kT_t[jk][64 * h : 64 * h + 64, pair, ksl],
                                        qT_t[j][64 * h : 64 * h + 64, pair, qoff:],
                                        start=True,
                                        stop=True,
                                    ).annotate("mm:s")
                            m0 = kt0 - 4 * j
                            if m0 < 0:  # both steps off-diagonal, full width
                                nc.scalar.activation(
                                    out=e, in_=psc, func=Exp
                                ).annotate("exp")
                            for s_ in range(2):
                                kt = kt0 + s_
                                m = kt - 4 * j
                                qoff = 128 * m if m >= 0 else 0
                                jk, km = kt // (DT // KT), kt % (DT // KT)
                                es = e[:, 2 * s_ : 2 * s_ + 2, :]
                                if m >= 0:
                                    nc.scalar.activation(
                                        out=es[:, :, qoff:],
                                        in_=psc[:, 2 * s_ : 2 * s_ + 2, qoff:],
                                        func=Exp,
                                    ).annotate("exp")
                                    # diagonal: the unmasked region's PV runs
                                    # straight off exp; the [128,128] causal
                                    # corner is masked then PV'd separately.
                                    # kt==0 is diagonal only at m==0, where
                                    # the full-width matmul carries start
                                    # (clears the whole bank) and the corner
                                    # accumulates with start=False.
                                    ro = qoff + KT
                                    if ro < DT:
                                        for h in range(2):
                                            nc.tensor.matmul(
                                                pvs[h][:, ro:],
                                                v_t[jk][:, km, 2 * pair + h, :],
                                                es[:, h, ro:],
                                                start=(kt == 0),
                                                stop=False,
                                            ).annotate("mm:pv")
                                    nc.vector.tensor_mul(
                                        out=es[:, :, qoff : qoff + KT],
                                        in0=es[:, :, qoff : qoff + KT],
                                        in1=msk_sb,
                                    ).annotate("mask")
                                    for h in range(2):
                                        nc.tensor.matmul(
                                            pvs[h][:, qoff : qoff + KT],
                                            v_t[jk][:, km, 2 * pair + h, :],
                                            es[:, h, qoff : qoff + KT],
                                            start=False,
                                            stop=(kt == nk - 1),
                                        ).annotate("mm:pv")
                                else:
                                    for h in range(2):
                                        nc.tensor.matmul(
                                            pvs[h][:, qoff:],
                                            v_t[jk][:, km, 2 * pair + h, :],
                                            es[:, h, qoff:],
                                            start=(kt == 0),
                                            stop=(kt == nk - 1),
                                        ).annotate("mm:pv")

                        cls.append(step2)

                    tail = j == NJ - 1 and pair == 1

                    def zcp(j=j, pair=pair, pvA=pvA, pvB=pvB, zz=zz, tail=tail):
                        # evacuate Z rows + unnormalized outputs NOW so the pv
                        # PSUM banks release quickly; head A goes through ACT,
                        # head B through DVE so the two evacuations overlap.
                        # At the very tail ACT is idle (exp done), so it also
                        # takes B's output copy, shortening the serial DVE
                        # chain in front of the final projection.
                        for half, pv in ((0, pvA), (1, pvB)):
                            row = 64 * half + 32 * pair
                            eng = nc.scalar.copy if half == 0 else nc.vector.tensor_copy
                            eng(
                                out=zz[row : row + 1, :],
                                in_=pv[HD : HD + 1, :],
                            ).annotate("zcp")
                            osl = o_t[j][half * 64 : half * 64 + 64, pair, :]
                            oeng = nc.scalar.copy if (tail or half == 0) else eng
                            oeng(out=osl, in_=pv[0:HD, :]).annotate("cp:o")

                    cls.append(zcp)

                    def norm_pair(j=j, pair=pair, zz=zz):
                        # per-pair normalization: pair0's runs while pair1's
                        # attention is still streaming; pair1's is deferred
                        # into the next q-tile (or the tail).  Z rows live at
                        # partition 64*half+32*pair; rows of the not-yet-done
                        # pair are memset 1.0 so the extra recip lanes are
                        # harmless, and each pair's matmul only references
                        # rows its own recip pass wrote.
                        rng = 65 if pair == 0 else 97
                        zr = rz_pool.tile([128, DT], F32, tag="zr")
                        nc.vector.reciprocal_approx_fast(
                            out=zr[0:rng, :], in_=zz[0:rng, :]
                        ).annotate("rz")
                        zrb = rz_pool.tile([128, DT], BF16, tag="zrb")
                        nc.vector.tensor_copy(
                            out=zrb[0:rng, :], in_=zr[0:rng, :]
                        ).annotate("rzb")
                        bz = tr_pool.tile([128, DT], F32, tag="tr")
                        nc.tensor.matmul(
                            bz,
                            sel_sb[0:rng, pair * 128 : (pair + 1) * 128],
                            zrb[0:rng, :],
                            start=True,
                            stop=True,
                        ).annotate("mm:bz")
                        for half in range(2):
                            osl = o_t[j][half * 64 : half * 64 + 64, pair, :]
                            nc.vector.tensor_mul(
                                out=osl, in0=osl, in1=bz[half * 64 : half * 64 + 64, :]
                            ).annotate("mul:o")

                    if pair == 0:
                        cls.append(norm_pair)
                    else:
                        norm = norm_pair

                return cls, norm

            def proj_closures(j):
                """fused projection (both pairs) — used for j < NJ-1 where
                there is no tail to hide."""
                cls = []
                for t_ in range(DT // KT):

                    def pj(j=j, t_=t_):
                        t0 = j * DT + t_ * 128
                        lsl = slice(t_ * 128, t_ * 128 + 128)
                        ysb = y_pool.tile([128, D], BF16, tag="y")
                        for n in range(2):
                            ps = tr_pool.tile([128, DT], F32, tag="tr")
                            nsl = slice(n * DT, n * DT + DT)
                            for ch in range(2):
                                nc.tensor.matmul(
                                    ps,
                                    o_t[j][:, ch, lsl],
                                    wp_sb[:, ch, nsl],
                                    start=(ch == 0),
                                    stop=(ch == 1),
                                ).annotate("mm:p")
                            nc.vector.tensor_copy(out=ysb[:, nsl], in_=ps).annotate(
                                "cp:y"
                            )
                        nc.sync.dma_start(y_d.ap()[t0 : t0 + 128, :], ysb).annotate(
                            "st:y"
                        )

                    cls.append(pj)
                return cls

            ysb_t = {}

            def proj0_closures(j):
                """pair0's half of the projection: runs while pair1's
                attention is still streaming."""
                cls = []
                for t_ in range(DT // KT):

                    def pj0(j=j, t_=t_):
                        lsl = slice(t_ * 128, t_ * 128 + 128)
                        ysb = y_pool.tile([128, D], BF16, tag="y", name=f"y{j}_{t_}")
                        ysb_t[(j, t_)] = ysb
                        for n in range(2):
                            ps = tr_pool.tile([128, DT], F32, tag="tr")
                            nsl = slice(n * DT, n * DT + DT)
                            nc.tensor.matmul(
                                ps,
                                o_t[j][:, 0, lsl],
                                wp_sb[:, 0, nsl],
                                start=True,
                                stop=True,
                            ).annotate("mm:p")
                            nc.vector.tensor_copy(out=ysb[:, nsl], in_=ps).annotate(
                                "cp:y"
                            )

                    cls.append(pj0)
                return cls

            def proj1_closures(j):
                cls = []
                for t_ in range(DT // KT):

                    def pj1(j=j, t_=t_):
                        t0 = j * DT + t_ * 128
                        lsl = slice(t_ * 128, t_ * 128 + 128)
                        ysb = ysb_t.pop((j, t_))
                        for n in range(2):
                            ps = tr_pool.tile([128, DT], F32, tag="tr")
                            nsl = slice(n * DT, n * DT + DT)
                            nc.tensor.matmul(
                                ps,
                                o_t[j][:, 1, lsl],
                                wp_sb[:, 1, nsl],
                                start=True,
                                stop=True,
                            ).annotate("mm:p")
                            nc.vector.tensor_add(
                                out=ysb[:, nsl], in0=ysb[:, nsl], in1=ps
                            ).annotate("ad:y")
                        nc.sync.dma_start(y_d.ap()[t0 : t0 + 128, :], ysb).annotate(
                            "st:y"
                        )

                    cls.append(pj1)
                return cls

            # ---- emission: QKV(0) plain, then per j interleave
            # attention(j) with QKV(j+1), proj-pair1(j-1) and proj-pair0(j) ----
            with nc.allow_low_precision(reason="bf16 activations"):
                for c in qkv_closures(0):
                    c()
                prev_norm = None
                for j in range(NJ):
                    attn, norm = attn_closures(j)
                    fill = []
                    if j + 1 < NJ:
                        qkv = qkv_closures(j + 1)
                        fill += qkv[:2]
                        if prev_norm is not None:
                            fill.append(prev_norm)
                            fill += proj_closures(j - 1)
                        fill += qkv[2:]
                    else:
                        fill.append(prev_norm)
                        fill += proj_closures(j - 1)
                        fill += proj0_closures(j)
                        prev_norm = None
                    prev_norm = norm
                    done = 0
                    for i, c in enumerate(attn):
                        c()
                        want = (i + 1) * len(fill) // len(attn)
                        while done < want:
                            fill[done]()
                            done += 1
                    while done < len(fill):
                        fill[done]()
                        done += 1
                prev_norm()
                for c in proj1_closures(NJ - 1):
                    c()

    return nc


def shard_inputs(x, W_kqv, b_kqv, W_proj, b_proj):
    """Build the 8 per-core input maps (host-side layout transforms)."""
    scale = 1.0 / np.sqrt(np.float32(HD))
    bf = lambda a: np.ascontiguousarray(a).astype(ml_dtypes.bfloat16)
    in_maps = []
    for c in range(N_CORES):
        b = c // (N_CORES // B)
        g = c % (N_CORES // B)
        gsl = slice(g * GF, (g + 1) * GF)
        wq = np.ascontiguousarray(W_kqv[:, gsl]) * scale
        wk = W_kqv[:, D + g * GF : D + (g + 1) * GF]
        wv = W_kqv[:, 2 * D + g * GF : 2 * D + (g + 1) * GF]
        bq = (b_kqv[gsl] * scale).reshape(2, 128).T
        bk = b_kqv[D + g * GF : D + (g + 1) * GF].reshape(2, 128).T
        bv = b_kqv[2 * D + g * GF : 2 * D + (g + 1) * GF].reshape(1, GF)
        # pre-tile to the SBUF layouts: one contiguous run per partition
        xt = np.asarray(x[b], dtype=np.float32).T  # [D, T]
        xt = xt.reshape(ND, 128, NJ, DT).transpose(1, 2, 0, 3)
        wtile = lambda w: np.asarray(w).reshape(ND, 128, GF).transpose(1, 0, 2)
        wtile_qk = lambda w: (
            np.asarray(w).reshape(ND, 128, 2, 128).transpose(1, 2, 0, 3)
        )
        in_maps.append(
            {
                "xT": bf(xt),
                "wq": bf(wtile_qk(wq)),
                "wk": bf(wtile_qk(wk)),
                "wv": bf(wtile(wv)),
                "wp": bf(
                    np.asarray(W_proj[gsl, :]).reshape(2, 128, D).transpose(1, 0, 2)
                ),
                "bq": np.ascontiguousarray(bq).astype(np.float32),
                "bk": np.ascontiguousarray(bk).astype(np.float32),
                "bv": bf(bv),
                "ones": np.ones((128, 128), dtype=ml_dtypes.bfloat16),
                "msk": _mask_tiles(),
                "sel": _sel_tiles(),
            }
        )
    return in_maps


def _sel_tiles():
    sel = np.zeros((128, 256), dtype=ml_dtypes.bfloat16)
    for p in range(2):
        for c in range(128):
            sel[64 * (c >= 64) + 32 * p, p * 128 + c] = 1.0
    return sel


def _mask_tiles():
    # triangular causal boundary for a diagonal [128k x 128q] corner:
    # keep where q_local >= k_local (c >= p), duplicated for both heads
    p = np.arange(128)[:, None]
    c = np.arange(KT)[None, :]
    m = (c >= p).astype(ml_dtypes.bfloat16)
    return np.ascontiguousarray(np.stack([m, m], axis=1))


def gather_outputs(results, b_proj):
    out = np.zeros((B, T, D), dtype=np.float32)
    for c in range(N_CORES):
        out[c // (N_CORES // B)] += np.asarray(results[c]["y"], dtype=np.float32)
    out += b_proj[None, None, :].astype(np.float32)
    return out


_NC_CACHE = {}


def _get_program(with_bias=False):
    if with_bias not in _NC_CACHE:
        nc = build_program(with_bias=with_bias)
        nc.finalize()  # runs Bacc passes (reg alloc, wait splitting)
        _NC_CACHE[with_bias] = nc
    return _NC_CACHE[with_bias]


def run(inputs, trace=False):
    """Run on the 8 NeuronCores; returns (out, BassKernelResults)."""
    from concourse import bass_utils

    # the v-bias matmul is only emitted when b_kqv's v-slice is nonzero
    # (it is all-zero for this model's initialization)
    with_bias = bool(np.any(np.asarray(inputs["b_kqv"][2 * D :])))
    nc = _get_program(with_bias)
    in_maps = shard_inputs(**inputs)
    res = bass_utils.run_bass_kernel_spmd(
        nc,
        in_maps,
        core_ids=list(range(N_CORES)),
        trace=trace,
        trace_cores=list(range(N_CORES)) if trace else None,
    )
    out = gather_outputs(res.results, inputs["b_proj"])
    return out, res


def kernel(**inputs):
    out, _ = run(inputs, trace=False)
    return out


# revision 73
# speedup vs baseline: 1.2240x; 1.2240x over previous
"""Causal MHA on 8 trn2 NeuronCores.

Sharding: core c handles batch b = c // 4 and head group g = c % 4
(heads 4g..4g+3).  Megatron-style TP: W_kqv column-split per head
group, W_proj row-split; the row-parallel all-reduce (sum of the 4
head-group partials per batch) happens on the host at gather time.

Per-core program (bf16 matmul operands, fp32 PSUM accumulation):
  - all DRAM inputs are pre-tiled on the host to the exact SBUF layouts
    (one contiguous run per partition) so every load is a single cheap
    DMA trigger, split across the two HWDGE queues (SP + ACT engine).
  - qT,kT produced directly in [feat, T] layout (lhsT=W tiles, rhs=xT
    tiles), v in [T, feat] layout, so no on-device transposes.
  - scores computed transposed, sT[k,q]; two heads packed into the PE
    array rows (K=64 each) via base-partition 0/64 -> concurrent MMs
    into the two banks of one [128,2,512] f32 PSUM tile.
  - causal trim: diagonal k-tiles only compute the valid q-range
    (N = 512-128m), so scores/exp/PV skip ~15% of work; the causal
    boundary is a [128,2,128] triangular-mask multiply (c >= p) per
    diagonal step, and PV of the already-valid region runs straight
    off exp so the mask never gates the bulk of the work.
  - exp on the scalar engine straight out of PSUM, one instruction
    covering both heads (1/sqrt(hd) folded into Wq on the host).
  - PV uses ones-augmented V ([k,65] lhsT) so PSUM row 64 accumulates
    the softmax denominator Z alongside the 64 output dims.
  - normalization per head-pair (pair0's runs under pair1's attention):
    Z rows evacuate via ACT/DVE split copies, 1/Z via the fast DVE
    reciprocal approximation, broadcast across 64 partitions with a
    tiny sel-matmul, then one DVE multiply per half.
  - projection of pair0's features for the last q-tile runs during
    pair1's attention (per-ch split with a DVE add), shrinking the tail.
  - emission interleaves QKV(j+1) and proj(j-1) matmuls into
    attention(j)'s exp-gated stream so the PE stays dense and warm;
    activations are per-j tiles so interleaved phases share no tiles.
  - the v-bias matmul is skipped when b_kqv is all-zero (it is, for
    this model); a with_bias program variant handles the general case.
"""

import sys

sys.path.insert(0, "/opt/trn_rl_repo")

import ml_dtypes
import numpy as np

import concourse.bass as bass
import concourse.tile as tile
from concourse import bacc, mybir

F32 = mybir.dt.float32
F32R = mybir.dt.float32r
BF16 = mybir.dt.bfloat16

B, T, D = 2, 2048, 1024
H, HD = 16, 64
N_CORES = 8
HPG = H // (N_CORES // B)  # heads per group = 4
GF = HPG * HD  # per-group feature width = 256
DT = 512  # t/q tile width
KT = 128  # k tile width
NJ = T // DT  # 4
ND = D // 128  # 8 contraction chunks

Exp = mybir.ActivationFunctionType.Exp
Ln = mybir.ActivationFunctionType.Ln


def build_program(num_devices=N_CORES, with_bias=False):
    nc = bacc.Bacc(
        "TRN2", target_bir_lowering=False, debug=False, num_devices=num_devices
    )
    # inputs are pre-tiled on the host to the exact SBUF layouts so each
    # load is ONE dma trigger with one contiguous run per partition
    # (HWDGE descriptor generation costs ~5ns/descriptor; 128 descs/DMA)
    xT_d = nc.dram_tensor("xT", [128, NJ, ND, DT], BF16, kind="ExternalInput")
    wq_d = nc.dram_tensor("wq", [128, 2, ND, 128], BF16, kind="ExternalInput")
    wk_d = nc.dram_tensor("wk", [128, 2, ND, 128], BF16, kind="ExternalInput")
    wv_d = nc.dram_tensor("wv", [128, ND, GF], BF16, kind="ExternalInput")
    wp_d = nc.dram_tensor("wp", [128, 2, D], BF16, kind="ExternalInput")
    bq_d = nc.dram_tensor("bq", [128, 2], F32, kind="ExternalInput")
    bk_d = nc.dram_tensor("bk", [128, 2], F32, kind="ExternalInput")
    bv_d = nc.dram_tensor("bv", [1, GF], BF16, kind="ExternalInput")
    ones_d = nc.dram_tensor("ones", [128, 128], BF16, kind="ExternalInput")
    msk_d = nc.dram_tensor("msk", [128, 2, KT], BF16, kind="ExternalInput")
    sel_d = nc.dram_tensor("sel", [128, 256], BF16, kind="ExternalInput")
    y_d = nc.dram_tensor("y", [T, D], BF16, kind="ExternalOutput")

    with tile.TileContext(nc) as tc:
        with (
            tc.tile_pool(name="singles", bufs=1) as singles,
            tc.tile_pool(name="ea", bufs=6) as e_pool,
            tc.tile_pool(name="rz", bufs=2) as rz_pool,
            tc.tile_pool(name="ysb", bufs=8) as y_pool,
            tc.tile_pool(name="tr", bufs=2, space="PSUM") as tr_pool,
            tc.tile_pool(name="sc", bufs=2, space="PSUM") as sc_pool,
            tc.tile_pool(name="pv", bufs=2, space="PSUM") as pv_pool,
        ):
            # ---- weights / constants resident in SBUF ----
            wq_sb = singles.tile([128, 2, ND, 128], BF16)
            wk_sb = singles.tile([128, 2, ND, 128], BF16)
            wv_sb = singles.tile([128, ND, GF], BF16)
            wp_sb = singles.tile([128, 2, D], BF16)
            bq_sb = singles.tile([128, 2], F32)
            bk_sb = singles.tile([128, 2], F32)
            bv_sb = singles.tile([1, GF], BF16)
            ones_sb = singles.tile([128, 128], BF16)
            msk_sb = singles.tile([128, 2, KT], BF16)
            sel_sb = singles.tile([128, 256], BF16)

            # per-j activation tiles (distinct tiles -> no false deps
            # between interleaved phases)
            qT_t = [singles.tile([128, 2, DT], BF16, tag=f"qT{j}", name=f"qT{j}") for j in range(NJ)]
            kT_t = [singles.tile([128, 2, DT], BF16, tag=f"kT{j}", name=f"kT{j}") for j in range(NJ)]
            v_t = [
                singles.tile([128, DT // KT, HPG, HD + 1], BF16, tag=f"v{j}", name=f"v{j}")
                for j in range(NJ)
            ]
            o_t = [singles.tile([128, 2, DT], BF16, tag=f"oT{j}", name=f"oT{j}") for j in range(NJ)]
            xt_t = [
                singles.tile([128, ND, DT], BF16, tag=f"xt{j}", name=f"xt{j}")
                for j in range(NJ)
            ]

            # ---- startup loads split across the two HWDGE queues (SP +
            # Activation engine): x tiles on SP, weights on ACT; the first
            # QKV accumulation starts once xt0's first half + wq land ----
            nc.sync.dma_start(xt_t[0][:, 0:4], xT_d.ap()[:, 0, 0:4]).annotate("ld:xt0")
            nc.sync.dma_start(xt_t[0][:, 4:8], xT_d.ap()[:, 0, 4:8]).annotate("ld:xt0")
            nc.scalar.dma_start(wq_sb[:, 0], wq_d.ap()[:, 0]).annotate("ld:wq")
            nc.scalar.dma_start(wk_sb[:, 0], wk_d.ap()[:, 0]).annotate("ld:wk")
            nc.scalar.dma_start(wq_sb[:, 1], wq_d.ap()[:, 1]).annotate("ld:wq")
            nc.scalar.dma_start(wk_sb[:, 1], wk_d.ap()[:, 1]).annotate("ld:wk")
            nc.scalar.dma_start(wv_sb, wv_d.ap()).annotate("ld:wv")
            nc.sync.dma_start(bq_sb, bq_d.ap()).annotate("ld:b")
            nc.sync.dma_start(bk_sb, bk_d.ap()).annotate("ld:b")
            nc.sync.dma_start(bv_sb, bv_d.ap()).annotate("ld:b")
            nc.sync.dma_start(ones_sb, ones_d.ap()).annotate("ld:b")
            nc.sync.dma_start(msk_sb, msk_d.ap()).annotate("ld:b")
            nc.sync.dma_start(sel_sb, sel_d.ap()).annotate("ld:b")
            nc.scalar.dma_start(xt_t[1], xT_d.ap()[:, 1]).annotate("ld:xt1")
            for j in range(2, NJ):
                nc.sync.dma_start(xt_t[j], xT_d.ap()[:, j]).annotate(f"ld:xt{j}")
            nc.scalar.dma_start(wp_sb, wp_d.ap()).annotate("ld:wp")
            ones1 = ones_sb[0:1, :]
            for j in range(NJ):
                nc.vector.tensor_copy(
                    out=v_t[j][:, :, :, HD],
                    in_=ones_sb[:, 0 : DT // KT * HPG].rearrange(
                        "p (a b) -> p a b", a=DT // KT
                    ),
                ).annotate("v:ones")

            def qkv_closures(j):
                """QKV production for t-tile j as a list of closures (q/k
                split per ch so pair0's attention can start after 2)."""
                cls = []
                for ch in range(2):
                    for w_sb, b_sb, dst in (
                        (wq_sb, bq_sb, qT_t[j]),
                        (wk_sb, bk_sb, kT_t[j]),
                    ):

                        def qk(j=j, ch=ch, w_sb=w_sb, b_sb=b_sb, dst=dst):
                            ps = tr_pool.tile([128, DT], F32, tag="tr")
                            for d in range(ND):
                                nc.tensor.matmul(
                                    ps,
                                    w_sb[:, ch, d, :],
                                    xt_t[j][:, d, :],
                                    start=(d == 0),
                                    stop=(d == ND - 1),
                                ).annotate("mm:qk")
                            nc.vector.tensor_scalar_add(
                                out=dst[:, ch, :],
                                in0=ps,
                                scalar1=b_sb[:, ch : ch + 1],
                            ).annotate("cp:qk")

                        cls.append(qk)

                for t_ in range(DT // KT):

                    def vv(j=j, t_=t_):
                        ps = tr_pool.tile([128, DT], F32, tag="tr")
                        ssl = slice(t_ * 128, t_ * 128 + 128)
                        for d in range(ND):
                            nc.tensor.matmul(
                                ps[:, 0:GF],
                                xt_t[j][:, d, ssl],
                                wv_sb[:, d, :],
                                start=(d == 0),
                                stop=(not with_bias) and (d == ND - 1),
                            ).annotate("mm:v")
                        if with_bias:
                            nc.tensor.matmul(
                                ps[:, 0:GF], ones1, bv_sb, start=False, stop=True
                            ).annotate("mm:vb")
                        nc.scalar.copy(
                            out=v_t[j][:, t_, :, 0:HD],
                            in_=ps[:, 0:GF].rearrange("p (h c) -> p h c", c=HD),
                        ).annotate("cp:v")

                    cls.append(vv)
                return cls

            def attn_closures(j):
                """Attention for q-tile j: per-(pair,kt) closures plus a
                normalize closure per pair."""
                nk = 4 * (j + 1)
                cls = []
                zz = rz_pool.tile([128, DT], F32, tag="zz", name=f"zz{j}")
                nc.vector.memset(zz, 1.0)
                for pair in range(2):
                    pvA = pv_pool.tile([HD + 1, DT], F32, tag="pv")
                    pvB = pv_pool.tile([HD + 1, DT], F32, tag="pv")

                    for kt in range(nk):
                        m = kt - 4 * j
                        qoff = 128 * m if m >= 0 else 0

                        def step(j=j, pair=pair, kt=kt, m=m, qoff=qoff,
                                 pvA=pvA, pvB=pvB, nk=nk):
                            jk, km = kt // (DT // KT), kt % (DT // KT)
                            ksl = slice(km * KT, km * KT + KT)
                            psc = sc_pool.tile([128, 2, DT], F32, tag="sc")
                            nc.tensor.matmul(
                                psc[:, 0, qoff:],
                                kT_t[jk][0:64, pair, ksl],
                                qT_t[j][0:64, pair, qoff:],
                                start=True,
                                stop=True,
                            ).annotate("mm:s")
                            nc.tensor.matmul(
                                psc[:, 1, qoff:],
                                kT_t[jk][64:128, pair, ksl],
                                qT_t[j][64:128, pair, qoff:],
                                start=True,
                                stop=True,
                            ).annotate("mm:s")
                            e = e_pool.tile([128, 2, DT], BF16, tag="e")
                            nc.scalar.activation(
                                out=e[:, :, qoff:], in_=psc[:, :, qoff:], func=Exp
                            ).annotate("exp")
                            pvs = (pvA, pvB)
                            if m >= 0:
                                # diagonal: the unmasked region's PV runs
                                # straight off exp; the [128,128] causal
                                # corner is masked then PV'd separately so
                                # the mask never gates the bulk of the work.
                                # kt==0 can only be diagonal at m==0, where
                                # the full-width matmul below carries start
                                # (clears the whole bank) and the corner
                                # accumulates with start=False.
                                ro = qoff + KT
                                if ro < DT:
                                    for h in range(2):
                                        nc.tensor.matmul(
                                            pvs[h][:, ro:],
                                            v_t[jk][:, km, 2 * pair + h, :],
                                            e[:, h, ro:],
                                            start=(kt == 0),
                                            stop=False,
                                        ).annotate("mm:pv")
                                nc.vector.tensor_mul(
                                    out=e[:, :, qoff : qoff + KT],
                                    in0=e[:, :, qoff : qoff + KT],
                                    in1=msk_sb,
                                ).annotate("mask")
                                for h in range(2):
                                    nc.tensor.matmul(
                                        pvs[h][:, qoff : qoff + KT],
                                        v_t[jk][:, km, 2 * pair + h, :],
                                        e[:, h, qoff : qoff + KT],
                                        start=False,
                                        stop=(kt == nk - 1),
                                    ).annotate("mm:pv")
                            else:
                                for h in range(2):
                                    nc.tensor.matmul(
                                        pvs[h][:, qoff:],
                                        v_t[jk][:, km, 2 * pair + h, :],
                                        e[:, h, qoff:],
                                        start=(kt == 0),
                                        stop=(kt == nk - 1),
                                    ).annotate("mm:pv")

                        cls.append(step)

                    tail = j == NJ - 1 and pair == 1

                    def zcp(j=j, pair=pair, pvA=pvA, pvB=pvB, zz=zz, tail=tail):
                        # evacuate Z rows + unnormalized outputs NOW so the pv
                        # PSUM banks release quickly; head A goes through ACT,
                        # head B through DVE so the two evacuations overlap.
                        # At the very tail ACT is idle (exp done), so it also
                        # takes B's output copy, shortening the serial DVE
                        # chain in front of the final projection.
                        for half, pv in ((0, pvA), (1, pvB)):
                            row = 64 * half + 32 * pair
                            eng = nc.scalar.copy if half == 0 else nc.vector.tensor_copy
                            eng(
                                out=zz[row : row + 1, :],
                                in_=pv[HD : HD + 1, :],
                            ).annotate("zcp")
                            osl = o_t[j][half * 64 : half * 64 + 64, pair, :]
                            oeng = nc.scalar.copy if (tail or half == 0) else eng
                            oeng(out=osl, in_=pv[0:HD, :]).annotate("cp:o")

                    cls.append(zcp)

                    def norm_pair(j=j, pair=pair, zz=zz):
                        # per-pair normalization: pair0's runs while pair1's
                        # attention is still streaming; pair1's is deferred
                        # into the next q-tile (or the tail).  Z rows live at
                        # partition 64*half+32*pair; rows of the not-yet-done
                        # pair are memset 1.0 so the extra recip lanes are
                        # harmless, and each pair's matmul only references
                        # rows its own recip pass wrote.
                        rng = 65 if pair == 0 else 97
                        zr = rz_pool.tile([128, DT], F32, tag="zr")
                        nc.vector.reciprocal_approx_fast(
                            out=zr[0:rng, :], in_=zz[0:rng, :]
                        ).annotate("rz")
                        zrb = rz_pool.tile([128, DT], BF16, tag="zrb")
                        nc.vector.tensor_copy(
                            out=zrb[0:rng, :], in_=zr[0:rng, :]
                        ).annotate("rzb")
                        bz = tr_pool.tile([128, DT], F32, tag="tr")
                        nc.tensor.matmul(
                            bz,
                            sel_sb[0:rng, pair * 128 : (pair + 1) * 128],
                            zrb[0:rng, :],
                            start=True,
                            stop=True,
                        ).annotate("mm:bz")
                        for half in range(2):
                            osl = o_t[j][half * 64 : half * 64 + 64, pair, :]
                            nc.vector.tensor_mul(
                                out=osl, in0=osl, in1=bz[half * 64 : half * 64 + 64, :]
                            ).annotate("mul:o")

                    if pair == 0:
                        cls.append(norm_pair)
                    else:
                        norm = norm_pair

                return cls, norm

            def proj_closures(j):
                """fused projection (both pairs) — used for j < NJ-1 where
                there is no tail to hide."""
                cls = []
                for t_ in range(DT // KT):

                    def pj(j=j, t_=t_):
                        t0 = j * DT + t_ * 128
                        lsl = slice(t_ * 128, t_ * 128 + 128)
                        ysb = y_pool.tile([128, D], BF16, tag="y")
                        for n in range(2):
                            ps = tr_pool.tile([128, DT], F32, tag="tr")
                            nsl = slice(n * DT, n * DT + DT)
                            for ch in range(2):
                                nc.tensor.matmul(
                                    ps,
                                    o_t[j][:, ch, lsl],
                                    wp_sb[:, ch, nsl],
                                    start=(ch == 0),
                                    stop=(ch == 1),
                                ).annotate("mm:p")
                            nc.vector.tensor_copy(out=ysb[:, nsl], in_=ps).annotate(
                                "cp:y"
                            )
                        nc.sync.dma_start(y_d.ap()[t0 : t0 + 128, :], ysb).annotate(
                            "st:y"
                        )

                    cls.append(pj)
                return cls

            ysb_t = {}

            def proj0_closures(j):
                """pair0's half of the projection: runs while pair1's
                attention is still streaming."""
                cls = []
                for t_ in range(DT // KT):

                    def pj0(j=j, t_=t_):
                        lsl = slice(t_ * 128, t_ * 128 + 128)
                        ysb = y_pool.tile([128, D], BF16, tag="y", name=f"y{j}_{t_}")
                        ysb_t[(j, t_)] = ysb
                        for n in range(2):
                            ps = tr_pool.tile([128, DT], F32, tag="tr")
                            nsl = slice(n * DT, n * DT + DT)
                            nc.tensor.matmul(
                                ps,
                                o_t[j][:, 0, lsl],
                                wp_sb[:, 0, nsl],
                                start=True,
                                stop=True,
                            ).annotate("mm:p")
                            nc.vector.tensor_copy(out=ysb[:, nsl], in_=ps).annotate(
                                "cp:y"
                            )

                    cls.append(pj0)
                return cls

            def proj1_closures(j):
                cls = []
                for t_ in range(DT // KT):

                    def pj1(j=j, t_=t_):
                        t0 = j * DT + t_ * 128
                        lsl = slice(t_ * 128, t_ * 128 + 128)
                        ysb = ysb_t.pop((j, t_))
                        for n in range(2):
                            ps = tr_pool.tile([128, DT], F32, tag="tr")
                            nsl = slice(n * DT, n * DT + DT)
                            nc.tensor.matmul(
                                ps,
                                o_t[j][:, 1, lsl],
                                wp_sb[:, 1, nsl],
                                start=True,
                                stop=True,
                            ).annotate("mm:p")
                            nc.vector.tensor_add(
                                out=ysb[:, nsl], in0=ysb[:, nsl], in1=ps
                            ).annotate("ad:y")
                        nc.sync.dma_start(y_d.ap()[t0 : t0 + 128, :], ysb).annotate(
                            "st:y"
                        )

                    cls.append(pj1)
                return cls

            # ---- emission: QKV(0) plain, then per j interleave
            # attention(j) with QKV(j+1), proj-pair1(j-1) and proj-pair0(j) ----
            with nc.allow_low_precision(reason="bf16 activations"):
                for c in qkv_closures(0):
                    c()
                prev_norm = None
                for j in range(NJ):
                    attn, norm = attn_closures(j)
                    fill = []
                    if j + 1 < NJ:
                        qkv = qkv_closures(j + 1)
                        fill += qkv[:2]
                        if prev_norm is not None:
                            fill.append(prev_norm)
                            fill += proj_closures(j - 1)
                        fill += qkv[2:]
                    else:
                        fill.append(prev_norm)
                        fill += proj_closures(j - 1)
                        fill += proj0_closures(j)
                        prev_norm = None
                    prev_norm = norm
                    done = 0
                    for i, c in enumerate(attn):
                        c()
                        want = (i + 1) * len(fill) // len(attn)
                        while done < want:
                            fill[done]()
                            done += 1
                    while done < len(fill):
                        fill[done]()
                        done += 1
                prev_norm()
                for c in proj1_closures(NJ - 1):
                    c()

    return nc


def shard_inputs(x, W_kqv, b_kqv, W_proj, b_proj):
    """Build the 8 per-core input maps (host-side layout transforms)."""
    scale = 1.0 / np.sqrt(np.float32(HD))
    bf = lambda a: np.ascontiguousarray(a).astype(ml_dtypes.bfloat16)
    in_maps = []
    for c in range(N_CORES):
        b = c // (N_CORES // B)
        g = c % (N_CORES // B)
        gsl = slice(g * GF, (g + 1) * GF)
        wq = np.ascontiguousarray(W_kqv[:, gsl]) * scale
        wk = W_kqv[:, D + g * GF : D + (g + 1) * GF]
        wv = W_kqv[:, 2 * D + g * GF : 2 * D + (g + 1) * GF]
        bq = (b_kqv[gsl] * scale).reshape(2, 128).T
        bk = b_kqv[D + g * GF : D + (g + 1) * GF].reshape(2, 128).T
        bv = b_kqv[2 * D + g * GF : 2 * D + (g + 1) * GF].reshape(1, GF)
        # pre-tile to the SBUF layouts: one contiguous run per partition
        xt = np.asarray(x[b], dtype=np.float32).T  # [D, T]
        xt = xt.reshape(ND, 128, NJ, DT).transpose(1, 2, 0, 3)
        wtile = lambda w: np.asarray(w).reshape(ND, 128, GF).transpose(1, 0, 2)
        wtile_qk = lambda w: (
            np.asarray(w).reshape(ND, 128, 2, 128).transpose(1, 2, 0, 3)
        )
        in_maps.append(
            {
                "xT": bf(xt),
                "wq": bf(wtile_qk(wq)),
                "wk": bf(wtile_qk(wk)),
                "wv": bf(wtile(wv)),
                "wp": bf(
                    np.asarray(W_proj[gsl, :]).reshape(2, 128, D).transpose(1, 0, 2)
                ),
                "bq": np.ascontiguousarray(bq).astype(np.float32),
                "bk": np.ascontiguousarray(bk).astype(np.float32),
                "bv": bf(bv),
                "ones": np.ones((128, 128), dtype=ml_dtypes.bfloat16),
                "msk": _mask_tiles(),
                "sel": _sel_tiles(),
            }
        )
    return in_maps


def _sel_tiles():
    sel = np.zeros((128, 256), dtype=ml_dtypes.bfloat16)
    for p in range(2):
        for c in range(128):
            sel[64 * (c >= 64) + 32 * p, p * 128 + c] = 1.0
    return sel


def _mask_tiles():
    # triangular causal boundary for a diagonal [128k x 128q] corner:
    # keep where q_local >= k_local (c >= p), duplicated for both heads
    p = np.arange(128)[:, None]
    c = np.arange(KT)[None, :]
    m = (c >= p).astype(ml_dtypes.bfloat16)
    return np.ascontiguousarray(np.stack([m, m], axis=1))


def gather_outputs(results, b_proj):
    out = np.zeros((B, T, D), dtype=np.float32)
    for c in range(N_CORES):
        out[c // (N_CORES // B)] += np.asarray(results[c]["y"], dtype=np.float32)
    out += b_proj[None, None, :].astype(np.float32)
    return out


_NC_CACHE = {}


def _get_program(with_bias=False):
    if with_bias not in _NC_CACHE:
        nc = build_program(with_bias=with_bias)
        nc.finalize()  # runs Bacc passes (reg alloc, wait splitting)
        _NC_CACHE[with_bias] = nc
    return _NC_CACHE[with_bias]


def run(inputs, trace=False):
    """Run on the 8 NeuronCores; returns (out, BassKernelResults)."""
    from concourse import bass_utils

    # the v-bias matmul is only emitted when b_kqv's v-slice is nonzero
    # (it is all-zero for this model's initialization)
    with_bias = bool(np.any(np.asarray(inputs["b_kqv"][2 * D :])))
    nc = _get_program(with_bias)
    in_maps = shard_inputs(**inputs)
    res = bass_utils.run_bass_kernel_spmd(
        nc,
        in_maps,
        core_ids=list(range(N_CORES)),
        trace=trace,
        trace_cores=list(range(N_CORES)) if trace else None,
    )
    out = gather_outputs(res.results, inputs["b_proj"])
    return out, res


def kernel(**inputs):
    out, _ = run(inputs, trace=False)
    return out
